# revision 10
# baseline (speedup 1.0000x reference)
"""GRU-D Trainium2 Bass kernel (v2 — latency-optimized serial chain).

Full-input contract: kernel(**inputs) takes the unsharded numpy inputs
(as produced by the reference setup) and returns the full [B, T-1, H]
output. Internally shards the batch across 8 NeuronCores (32 rows each),
runs a Tile/Bass kernel per core (SPMD, no collectives), and gathers.

Layout notes (per core, "frame" = time-major columns):
  frame col f = (t-1)*32 + b   for step t in 1..T-1, batch row b in 0..31
  x_t     [64, T*32]   col t*32+b = x[b, t, d]   (xl(f)=x_t[:,f], xn(f)=x_t[:,f+32])
  mask_t  [64, NF]     col f = mask[b, t, d]
  delta_t [1,  NF]     col f = delta[b, t]
  out_t   [128, NF]    col f = 2*h_t[b, h]   (state kept doubled; halved on host)

Math restructuring (exact):
  sigmoid(u) = (1 + tanh(u/2)) / 2  -> whole kernel uses only Tanh/Exp
  (one ACT table set => no table reloads).
  ghk = gh/2 = min(exp(-(w*d+b)), 1)/2 ; hp_t = ghk_t (.) Hd_{t-1} = gh (.) h_{t-1}
  With y_z = tanh((s_z+bz)/2), y_r = tanh((s_r+br)/2):
    rh2 = (y_r+1)(.)hp = 2 r (.) hp ; ph += (Wh_h/2)·rh2
    Hd_t = 2h_t = (1+y_z)(.)ht + (1-y_z)(.)hp
    hp_{t+1} = ghk'(.)Hd_t = w_t + nq_t  where
      w_t  = p_t (.) ht,           p_t  = ghk' (.) (1+y_z)     [ready early]
      nq_t = q1_t (.) (-ghk'),     q1_t = (y_z-1) (.) hp       [ready early]
  Critical path per step:  w -> [PE pr] -> [ACT yr] -> [DVE rh2] ->
    [PE ph] -> [ACT ht] -> [DVE w'] -> ...
  The nq_t contributions to next step's pr/pz PSUM are accumulated by
  separate matmuls that run BEFORE ht_t is known (latency hiding), and
  the output blend Hd_t runs on the (otherwise idle) Pool engine.
"""

import numpy as np
from contextlib import ExitStack

B, T, D, H = 256, 512, 64, 128
NCORES = 8
BB = B // NCORES  # 32 batch rows per core
BLK = 16          # time steps per PSUM bank block

_cache = {}


def _build(ts):
    """Build the Bass module for a scan of `ts` steps. Returns nc."""
    import concourse.bass as bass
    import concourse.bacc as bacc
    import concourse.tile as tile
    import concourse.mybir as mybir
    from concourse.mybir import AluOpType as alu
    from concourse.mybir import ActivationFunctionType as actf

    f32 = mybir.dt.float32
    f32r = mybir.dt.float32r
    NF = ts * BB
    NBLK = (ts + BLK - 1) // BLK
    RING = min(128, NBLK * BLK)  # out ring length in steps (multiple of BLK)

    nc = bacc.Bacc("TRN2", target_bir_lowering=False, debug=False)

    dx = nc.dram_tensor("x_t", [D, (ts + 1) * BB], f32, kind="ExternalInput")
    dm = nc.dram_tensor("mask_t", [D, NF], f32, kind="ExternalInput")
    dd = nc.dram_tensor("delta_t", [1, NF], f32, kind="ExternalInput")
    # weights: bulk (xt|m rows) and chain (h rows); whh pre-scaled by 0.5
    dwz_xm = nc.dram_tensor("wz_xm", [2 * D, H], f32, kind="ExternalInput")
    dwr_xm = nc.dram_tensor("wr_xm", [2 * D, H], f32, kind="ExternalInput")
    dwh_xm = nc.dram_tensor("wh_xm", [2 * D, H], f32, kind="ExternalInput")
    dwz_h = nc.dram_tensor("wz_h", [H, H], f32, kind="ExternalInput")
    dwr_h = nc.dram_tensor("wr_h", [H, H], f32, kind="ExternalInput")
    dwh_h = nc.dram_tensor("wh_h2", [H, H], f32, kind="ExternalInput")
    # biases: bz/2, br/2, bh ; gate params: w rows, -b cols ; X_mean col
    dbz = nc.dram_tensor("bz2", [H, 1], f32, kind="ExternalInput")
    dbr = nc.dram_tensor("br2", [H, 1], f32, kind="ExternalInput")
    dbh = nc.dram_tensor("bh", [H, 1], f32, kind="ExternalInput")
    dwgh = nc.dram_tensor("w_gh", [1, H], f32, kind="ExternalInput")
    dbgh = nc.dram_tensor("nb_gh", [H, 1], f32, kind="ExternalInput")
    dwgx = nc.dram_tensor("w_gx", [1, D], f32, kind="ExternalInput")
    dbgx = nc.dram_tensor("nb_gx", [D, 1], f32, kind="ExternalInput")
    dxmean = nc.dram_tensor("x_mean", [D, 1], f32, kind="ExternalInput")
    dout = nc.dram_tensor("out_t", [H, NF], f32, kind="ExternalOutput")

    def ncols_of(k):
        return min(BLK * BB, NF - k * BLK * BB)

    with tile.TileContext(nc) as tc, ExitStack() as ctx:
        const = ctx.enter_context(tc.tile_pool(name="const", bufs=1))

        def cload(dram, shape):
            t = const.tile(shape, f32, tag=f"c_{dram.name}")
            nc.sync.dma_start(t[:], dram.ap())
            return t

        w_zxm = cload(dwz_xm, [2 * D, H])
        w_rxm = cload(dwr_xm, [2 * D, H])
        w_hxm = cload(dwh_xm, [2 * D, H])
        w_zh = cload(dwz_h, [H, H])
        w_rh = cload(dwr_h, [H, H])
        w_hh = cload(dwh_h, [H, H])
        bz2 = cload(dbz, [H, 1])
        br2 = cload(dbr, [H, 1])
        bh = cload(dbh, [H, 1])
        w_gh = cload(dwgh, [1, H])
        nb_gh = cload(dbgh, [H, 1])
        w_gx = cload(dwgx, [1, D])
        nb_gx = cload(dbgx, [D, 1])
        xmean = cload(dxmean, [D, 1])

        hp0 = const.tile([H, BB], f32, tag="c_hp0")
        nc.vector.memset(hp0[:], 0.0)

        outring = ctx.enter_context(tc.tile_pool(name="outring", bufs=1)).tile(
            [H, RING * BB], f32
        )

        # --- pools ---
        dpool = ctx.enter_context(tc.tile_pool(name="dpool", bufs=2))
        gpsum = ctx.enter_context(
            tc.tile_pool(name="gpsum", bufs=1, space=bass.MemorySpace.PSUM)
        )
        epool = ctx.enter_context(tc.tile_pool(name="epool", bufs=2))
        ghpool = ctx.enter_context(tc.tile_pool(name="ghpool", bufs=2))
        gxpool = ctx.enter_context(tc.tile_pool(name="gxpool", bufs=2))
        xpool = ctx.enter_context(tc.tile_pool(name="xpool", bufs=2))
        rhspool = ctx.enter_context(tc.tile_pool(name="rhspool", bufs=2))
        tpool = ctx.enter_context(tc.tile_pool(name="tpool", bufs=2))
        psz = ctx.enter_context(
            tc.tile_pool(name="psz", bufs=2, space=bass.MemorySpace.PSUM)
        )
        psr = ctx.enter_context(
            tc.tile_pool(name="psr", bufs=2, space=bass.MemorySpace.PSUM)
        )
        psh = ctx.enter_context(
            tc.tile_pool(name="psh", bufs=2, space=bass.MemorySpace.PSUM)
        )
        spool = ctx.enter_context(tc.tile_pool(name="spool", bufs=4))

        # ---------- per-block state handles ----------
        blk_state = {}

        def emit_dma(k):
            """Start input DMAs for block k; create its tiles."""
            if k >= NBLK or k in blk_state:
                return
            c0 = k * BLK * BB
            ncols = ncols_of(k)
            dch = dpool.tile([1, ncols], f32, tag="dch")
            nc.sync.dma_start(dch[:], dd.ap()[:, c0 : c0 + ncols])
            xch = xpool.tile([D, ncols + BB], f32, tag="xch")
            nc.sync.dma_start(xch[:], dx.ap()[:, c0 : c0 + ncols + BB])
            rhs = rhspool.tile([2 * D, ncols], f32, tag="rhs")
            nc.sync.dma_start(rhs[D : 2 * D, :], dm.ap()[:, c0 : c0 + ncols])
            mk = tpool.tile([D, ncols], f32, tag="mk")
            nc.sync.dma_start(mk[:], dm.ap()[:, c0 : c0 + ncols])
            blk_state[k] = {
                "dch": dch, "xch": xch, "rhs": rhs, "mk": mk, "ncols": ncols
            }

        def emit_gmm(k):
            """Decay-gate matmuls for block k (PE, into gpsum)."""
            st = blk_state[k]
            ncols = st["ncols"]
            pg = gpsum.tile([H, ncols], f32, tag="pgh")
            nc.tensor.matmul(pg[:], w_gh[:], st["dch"][:], start=True, stop=True)
            pgx = gpsum.tile([D, ncols], f32, tag="pgx")
            nc.tensor.matmul(pgx[:], w_gx[:], st["dch"][:], start=True, stop=True)
            st["pg"] = pg
            st["pgx"] = pgx

        def emit_eg(k, half):
            """exp() for the h-decay gate, one half per call (Act)."""
            st = blk_state[k]
            ncols = st["ncols"]
            h1 = (ncols // 2) // BB * BB
            if half == 0:
                eg = epool.tile([H, ncols], f32, tag="eg")
                st["eg"] = eg
                nc.scalar.activation(
                    eg[:, 0:h1], st["pg"][:, 0:h1], actf.Exp,
                    bias=nb_gh[:], scale=-1.0,
                )
            else:
                nc.scalar.activation(
                    st["eg"][:, h1:ncols], st["pg"][:, h1:ncols], actf.Exp,
                    bias=nb_gh[:], scale=-1.0,
                )

        def emit_egx(k, half):
            st = blk_state[k]
            ncols = st["ncols"]
            h1 = (ncols // 2) // BB * BB
            if half == 0:
                egx = epool.tile([D, ncols], f32, tag="egx")
                st["egx"] = egx
                nc.scalar.activation(
                    egx[:, 0:h1], st["pgx"][:, 0:h1], actf.Exp,
                    bias=nb_gx[:], scale=-1.0,
                )
            else:
                nc.scalar.activation(
                    st["egx"][:, h1:ncols], st["pgx"][:, h1:ncols], actf.Exp,
                    bias=nb_gx[:], scale=-1.0,
                )

        def emit_ghk(k):
            """ghk = min(eg,1)*0.5 and nghk = min(eg,1)*(-0.5)  (Pool)."""
            st = blk_state[k]
            ncols = st["ncols"]
            ghk = ghpool.tile([H, ncols], f32, tag="ghk")
            nc.gpsimd.tensor_scalar(ghk[:], st["eg"][:], 1.0, 0.5, alu.min, alu.mult)
            st["ghk"] = ghk

        def emit_nghk(k):
            st = blk_state[k]
            ncols = st["ncols"]
            nghk = ghpool.tile([H, ncols], f32, tag="nghk")
            nc.gpsimd.tensor_scalar(
                nghk[:], st["eg"][:], 1.0, -0.5, alu.min, alu.mult
            )
            st["nghk"] = nghk

        def emit_gxk(k):
            st = blk_state[k]
            ncols = st["ncols"]
            gxk = gxpool.tile([D, ncols], f32, tag="gxk")
            nc.gpsimd.tensor_scalar(gxk[:], st["egx"][:], 1.0, None, alu.min)
            st["gxk"] = gxk

        def emit_xprep(k, i):
            """One of 6 Pool ops building rhs[0:D] = xt for block k."""
            st = blk_state[k]
            ncols = st["ncols"]
            xch, rhs = st["xch"], st["rhs"]
            xl = xch[:, 0:ncols]
            xn = xch[:, BB : ncols + BB]
            mk = st["mk"][:]
            if i == 0:
                a1 = tpool.tile([D, ncols], f32, tag="a1")
                nc.gpsimd.tensor_scalar(a1[:], xl, xmean[:], None, alu.subtract)
                st["a1"] = a1
            elif i == 1:
                b1 = tpool.tile([D, ncols], f32, tag="b1")
                nc.gpsimd.tensor_mul(b1[:], st["gxk"][:], st["a1"][:])
                st["b1"] = b1
            elif i == 2:
                c1 = tpool.tile([D, ncols], f32, tag="c1")
                nc.gpsimd.tensor_scalar(c1[:], st["b1"][:], xmean[:], None, alu.add)
                st["c1"] = c1
            elif i == 3:
                e1 = tpool.tile([D, ncols], f32, tag="e1")
                nc.gpsimd.tensor_sub(e1[:], xn, st["c1"][:])
                st["e1"] = e1
            elif i == 4:
                f1 = tpool.tile([D, ncols], f32, tag="f1")
                nc.gpsimd.tensor_mul(f1[:], mk, st["e1"][:])
                st["f1"] = f1
            elif i == 5:
                nc.gpsimd.tensor_add(rhs[0:D, :], st["f1"][:], st["c1"][:])

        def emit_bulk(k, gate):
            """Bulk preactivation matmul for one gate, fp32, 2 pieces (PE)."""
            st = blk_state[k]
            ncols = st["ncols"]
            h1 = (ncols // 2) // BB * BB
            pool, wmat = {
                "r": (psr, w_rxm),
                "z": (psz, w_zxm),
                "h": (psh, w_hxm),
            }[gate]
            pt = pool.tile([H, ncols], f32, tag="p" + gate)
            rhs = st["rhs"]
            # start=True resets the whole PSUM bank: only the FIRST piece
            # may carry it; later pieces accumulate onto the zeroed bank.
            nc.tensor.matmul(
                pt[:, 0:h1], wmat[:], rhs[:, 0:h1],
                start=True, stop=False, skip_group_check=True,
            )
            nc.tensor.matmul(
                pt[:, h1:ncols], wmat[:], rhs[:, h1:ncols],
                start=False, stop=True, skip_group_check=True,
            )
            st["ps" + gate] = pt

        def prep_all(k):
            """Emit the whole prep for block k contiguously (prologue only)."""
            emit_gmm(k)
            emit_eg(k, 0)
            emit_eg(k, 1)
            emit_egx(k, 0)
            emit_egx(k, 1)
            emit_ghk(k)
            emit_nghk(k)
            emit_gxk(k)
            for i in range(6):
                emit_xprep(k, i)
            for g in ("r", "z", "h"):
                emit_bulk(k, g)

        # ---------- prologue ----------
        emit_dma(0)
        prep_all(0)
        emit_dma(1)

        # carried across steps: w/nq tiles of the previous step
        carry = {"w": None, "nq": None, "hp": hp0}

        for k in range(NBLK):
            st = blk_state[k]
            ncols = st["ncols"]
            nsteps = ncols // BB
            pr, pz, ph = st["psr"], st["psz"], st["psh"]
            ghk, nghk = st["ghk"], st["nghk"]

            for s in range(nsteps):
                t = k * BLK + s + 1  # step index 1..ts
                last = t == ts
                rpos = (t - 1) % RING
                sl = slice(s * BB, (s + 1) * BB)
                osl = slice(rpos * BB, (rpos + 1) * BB)

                # ghk' (= decay for step t+1) slice
                if not last:
                    if s + 1 < nsteps:
                        ghn = ghk[:, (s + 1) * BB : (s + 2) * BB]
                        nghn = nghk[:, (s + 1) * BB : (s + 2) * BB]
                    else:
                        ghn = blk_state[k + 1]["ghk"][:, 0:BB]
                        nghn = blk_state[k + 1]["nghk"][:, 0:BB]

                # -- PE: finish this step's pr/pz with the carried w
                #    (nq contribution was already accumulated last step)
                if carry["w"] is not None:
                    nc.tensor.matmul(
                        pr[:, sl], w_rh[:], carry["w"][:], start=False, stop=True,
                        skip_group_check=True,
                    )
                    nc.tensor.matmul(
                        pz[:, sl], w_zh[:], carry["w"][:], start=False, stop=True,
                        skip_group_check=True,
                    )

                # -- interleaved prep for block k+1 --
                if k + 1 < NBLK:
                    if s == 0:
                        emit_dma(k + 2)
                    elif s == 1:
                        emit_gmm(k + 1)
                    elif s == 2:
                        emit_xprep(k + 1, 0)
                    elif s == 3:
                        emit_eg(k + 1, 0)
                    elif s == 4:
                        emit_eg(k + 1, 1)
                    elif s == 5:
                        emit_ghk(k + 1)
                    elif s == 6:
                        emit_egx(k + 1, 0)
                        emit_nghk(k + 1)
                    elif s == 7:
                        emit_egx(k + 1, 1)
                    elif s == 8:
                        emit_gxk(k + 1)
                    elif 9 <= s <= 12:
                        emit_xprep(k + 1, s - 8)
                    elif s == 13:
                        emit_xprep(k + 1, 5)
                        emit_bulk(k + 1, "r")
                    elif s == 14:
                        emit_bulk(k + 1, "z")
                    elif s == 15:
                        emit_bulk(k + 1, "h")

                # -- ACT: gates
                yr = spool.tile([H, BB], f32, tag="yr")
                nc.scalar.activation(yr[:], pr[:, sl], actf.Tanh, bias=br2[:], scale=0.5)
                yz = spool.tile([H, BB], f32, tag="yz")
                nc.scalar.activation(yz[:], pz[:, sl], actf.Tanh, bias=bz2[:], scale=0.5)

                hp = carry["hp"]

                # -- DVE: candidate gate input (critical)
                rh2 = spool.tile([H, BB], f32, tag="rh2")
                nc.vector.scalar_tensor_tensor(
                    rh2[:], yr[:], 1.0, hp[:], alu.add, alu.mult
                )
                # -- DVE off-path: q1, nq, p
                q1 = spool.tile([H, BB], f32, tag="q1")
                nc.vector.scalar_tensor_tensor(
                    q1[:], yz[:], 1.0, hp[:], alu.subtract, alu.mult
                )
                if not last:
                    nq = spool.tile([H, BB], f32, tag="nq")
                    nc.vector.tensor_mul(nq[:], q1[:], nghn)
                    p = spool.tile([H, BB], f32, tag="p")
                    nc.vector.scalar_tensor_tensor(
                        p[:], yz[:], 1.0, ghn, alu.add, alu.mult
                    )

                # -- PE: candidate matmul (critical)
                nc.tensor.matmul(
                    ph[:, sl], w_hh[:], rh2[:], start=False, stop=True,
                    skip_group_check=True,
                )

                # -- PE: pre-accumulate nq into NEXT step's pr/pz
                if not last:
                    if s + 1 < nsteps:
                        nsl = slice((s + 1) * BB, (s + 2) * BB)
                        npr, npz = pr, pz
                    else:
                        nsl = slice(0, BB)
                        npr = blk_state[k + 1]["psr"]
                        npz = blk_state[k + 1]["psz"]
                    nc.tensor.matmul(
                        npr[:, nsl], w_rh[:], nq[:], start=False, stop=True,
                        skip_group_check=True,
                    )
                    nc.tensor.matmul(
                        npz[:, nsl], w_zh[:], nq[:], start=False, stop=True,
                        skip_group_check=True,
                    )

                # -- ACT: candidate tanh (critical)
                ht = spool.tile([H, BB], f32, tag="ht")
                nc.scalar.activation(ht[:], ph[:, sl], actf.Tanh, bias=bh[:], scale=1.0)

                # -- DVE: w = p (.) ht (critical), hp' = w + nq (off-path)
                if not last:
                    w = spool.tile([H, BB], f32, tag="w")
                    nc.vector.tensor_mul(w[:], p[:], ht[:])
                    hpn = spool.tile([H, BB], f32, tag="hp")
                    nc.vector.tensor_add(hpn[:], w[:], nq[:])
                    carry = {"w": w, "nq": nq, "hp": hpn}

                # -- output blend Hd = (1+yz)ht - (yz-1)hp  (off-path):
                #    m12 on DVE (stt unsupported on Pool), final sub on Pool
                m12 = spool.tile([H, BB], f32, tag="m12")
                nc.vector.scalar_tensor_tensor(
                    m12[:], yz[:], 1.0, ht[:], alu.add, alu.mult
                )
                nc.gpsimd.tensor_sub(outring[:, osl], m12[:], q1[:])

            # ---- flush this block's outputs to DRAM
            c0 = k * BLK * BB
            rk = (k * BLK) % RING
            nc.sync.dma_start(
                dout.ap()[:, c0 : c0 + ncols],
                outring[:, rk * BB : rk * BB + ncols],
            )

    return nc


def _prep_shared(inputs):
    f = np.float32
    Wz, Wr, Wh = inputs["Wz"], inputs["Wr"], inputs["Wh"]

    def xm(W):
        return np.ascontiguousarray(
            np.concatenate([W[:D], W[D + H :]], axis=0), dtype=f
        )

    def hh(W):
        return np.ascontiguousarray(W[D : D + H], dtype=f)

    return {
        "wz_xm": xm(Wz),
        "wr_xm": xm(Wr),
        "wh_xm": xm(Wh),
        "wz_h": hh(Wz),
        "wr_h": hh(Wr),
        "wh_h2": np.ascontiguousarray(0.5 * hh(Wh), dtype=f),
        "bz2": np.ascontiguousarray(0.5 * inputs["bz"].reshape(H, 1), dtype=f),
        "br2": np.ascontiguousarray(0.5 * inputs["br"].reshape(H, 1), dtype=f),
        "bh": np.ascontiguousarray(inputs["bh"].reshape(H, 1), dtype=f),
        "w_gh": np.ascontiguousarray(inputs["w_gh"].reshape(1, H), dtype=f),
        "nb_gh": np.ascontiguousarray(-inputs["b_gh"].reshape(H, 1), dtype=f),
        "w_gx": np.ascontiguousarray(inputs["w_gx"].reshape(1, D), dtype=f),
        "nb_gx": np.ascontiguousarray(-inputs["b_gx"].reshape(D, 1), dtype=f),
        "x_mean": np.ascontiguousarray(inputs["X_mean"].reshape(D, 1), dtype=f),
    }


def _run(inputs, ts, trace=False):
    """Run the scan for `ts` steps (uses x[:, :ts+1] etc). Returns
    ([B, ts, H] float32 output, exec_time_ns or None)."""
    from concourse.bass_utils import run_bass_kernel_spmd

    if ts not in _cache:
        nc = _build(ts)
        if not nc.is_finalized():
            nc.finalize()
        _cache[ts] = nc
    nc = _cache[ts]

    shared = _prep_shared(inputs)
    x = np.asarray(inputs["x"], dtype=np.float32)
    mask = np.asarray(inputs["mask"], dtype=np.float32)
    delta = np.asarray(inputs["delta_t"], dtype=np.float32)

    in_maps = []
    for c in range(NCORES):
        bs = slice(c * BB, (c + 1) * BB)
        xc = np.ascontiguousarray(
            x[bs, : ts + 1].transpose(2, 1, 0).reshape(D, (ts + 1) * BB)
        )
        mc = np.ascontiguousarray(
            mask[bs, 1 : ts + 1].transpose(2, 1, 0).reshape(D, ts * BB)
        )
        dc = np.ascontiguousarray(delta[bs, 1 : ts + 1].T.reshape(1, ts * BB))
        in_maps.append({"x_t": xc, "mask_t": mc, "delta_t": dc, **shared})

    res = run_bass_kernel_spmd(
        nc, in_maps, core_ids=list(range(NCORES)), trace=trace
    )
    outs = []
    for c in range(NCORES):
        o = res.results[c]["out_t"]  # [H, ts*BB], holds 2*h
        outs.append(o.reshape(H, ts, BB).transpose(2, 1, 0))
    full = np.concatenate(outs, axis=0) * np.float32(0.5)
    return np.ascontiguousarray(full, dtype=np.float32), res.exec_time_ns


def kernel(**inputs):
    out, _ = _run(inputs, T - 1)
    return out
